# revision 7
# baseline (speedup 1.0000x reference)
"""Trainium2 Bass kernel for nn_DenoiseQNN (conv -> global avgpool -> 4-qubit
quantum circuit -> MLP decoder), data-parallel over 8 NeuronCores.

Math folding (validated against the jax reference on host):
  * conv(3->4, 3x3, SAME) followed by global mean is LINEAR in x, and depends
    on x only through 27 per-sample scalars: per input channel the total sum,
    the 4 border row/col sums, and the 4 corner pixels (inclusion-exclusion
    over the 9 kernel taps). pooled = F @ Weff (+ conv_b via a const-1
    feature).
  * the quantum state after the per-sample RY embedding layer is the real
    product state s_ry[j] = prod_w (cos(p_w/2) if bit_w(j)==0 else sin(p_w/2)).
    The remaining RX layers + CNOT rings use only the shared q_weights, so
    they form a fixed complex 16x16 matrix M. <Z_w> = s^T G_w s with
    G_w = Re(M)^T diag(Z_w) Re(M) + Im(M)^T diag(Z_w) Im(M), and the first MLP
    layer folds in: hpre_m = s^T H_m s, H_m = sum_w w1[m,w] G_w.
  * out = relu(hpre + b1) @ w2.T + b2 -> [B, 3072].

Device pipeline per 128-sample tile (batch on partitions):
  DMA x tile [128, 3072] -> DVE reductions build F [128, 32] -> 4 fused
  multiply-reduce ops give pooled [128, 4] -> ScalarE Sin activations give
  cos/sin -> DVE broadcast-multiplies build s16 and the outer products
  P2 [128, 256] -> TensorE transposes P2 -> two matmuls against H (lhsT)
  give hpre [128m, 128b] -> ScalarE relu+bias -> TensorE [128,128]x[128,3072]
  matmul against w2^T -> ScalarE PSUM->SBUF copy -> DMA out.

Traffic is 2 x 201 MB (read x, write out) over 8 cores; everything else is
tiny, so the kernel is HBM-bound as intended.
"""

import math
from contextlib import ExitStack

import numpy as np

import concourse.bass as bass
import concourse.mybir as mybir
import concourse.tile as tile
from concourse import bacc
from concourse.bass_utils import run_bass_kernel_spmd

N_CORES = 8
B_FULL = 16384
B_SHARD = B_FULL // N_CORES  # 2048
P = 128
D = 3072  # 3*32*32
N_QUBITS = 4
DIM = 16
F32 = mybir.dt.float32
HALF_PI = math.pi / 2.0


# ---------------------------------------------------------------------------
# Host-side parameter folding
# ---------------------------------------------------------------------------

def _feature_weights(conv_w: np.ndarray, conv_b: np.ndarray) -> np.ndarray:
    """Weff [32, 4]: pooled = F @ Weff with the device feature layout
    F = [S(3), R0(3), R31(3), C0(3), C31(3), corners(3x2x2), 1, pad(4)]."""
    W = np.zeros((32, N_QUBITS), np.float64)
    cw = conv_w.astype(np.float64)
    for o in range(N_QUBITS):
        for i in range(3):
            for dh in range(3):
                for dw in range(3):
                    c = cw[o, i, dh, dw]
                    W[0 + i, o] += c                     # total sum
                    if dh == 2:
                        W[3 + i, o] -= c                 # row 0 excluded
                    if dh == 0:
                        W[6 + i, o] -= c                 # row 31 excluded
                    if dw == 2:
                        W[9 + i, o] -= c                 # col 0 excluded
                    if dw == 0:
                        W[12 + i, o] -= c                # col 31 excluded
                    # corners (i, r, c): r,c in {0,31}
                    if (dh, dw) == (2, 2):
                        W[15 + i * 4 + 0, o] += c        # x[0,0]
                    if (dh, dw) == (2, 0):
                        W[15 + i * 4 + 1, o] += c        # x[0,31]
                    if (dh, dw) == (0, 2):
                        W[15 + i * 4 + 2, o] += c        # x[31,0]
                    if (dh, dw) == (0, 0):
                        W[15 + i * 4 + 3, o] += c        # x[31,31]
    W /= 1024.0
    W[27, :] = conv_b.astype(np.float64)
    return W.astype(np.float32)


def _quantum_fixed_matrix(q_weights: np.ndarray) -> np.ndarray:
    """M [16,16] complex: the fixed post-RY linear map (RX layers + CNOT rings)."""
    M = np.eye(DIM, dtype=np.complex128)

    def apply_1q(Mat, U, wire):
        T = Mat.reshape(2**wire, 2, 2 ** (N_QUBITS - 1 - wire), DIM)
        T = np.einsum("ij,ajcb->aicb", U, T)
        return T.reshape(DIM, DIM)

    idx = np.arange(DIM)
    perms = []
    for c in range(N_QUBITS):
        t = (c + 1) % N_QUBITS
        mc = 1 << (N_QUBITS - 1 - c)
        mt = 1 << (N_QUBITS - 1 - t)
        perms.append(np.where(idx & mc, idx ^ mt, idx))

    for layer in range(q_weights.shape[0]):
        for w in range(N_QUBITS):
            th = float(q_weights[layer, w]) * 0.5
            cc = np.cos(th)
            ss = -1j * np.sin(th)
            M = apply_1q(M, np.array([[cc, ss], [ss, cc]]), w)
        for w in range(N_QUBITS):
            M = M[perms[w], :]
    return M


def _quadratic_forms(q_weights: np.ndarray, w1: np.ndarray) -> np.ndarray:
    """H [128, 16, 16]: hpre[m] = s^T H_m s."""
    M = _quantum_fixed_matrix(q_weights)
    bits = (np.arange(DIM)[None, :] >> (N_QUBITS - 1 - np.arange(N_QUBITS)[:, None])) & 1
    Z = 1.0 - 2.0 * bits
    Re, Im = M.real, M.imag
    G = np.einsum("wj,jk,jl->wkl", Z, Re, Re) + np.einsum("wj,jk,jl->wkl", Z, Im, Im)
    return np.einsum("mw,wkl->mkl", w1.astype(np.float64), G).astype(np.float32)


# ---------------------------------------------------------------------------
# Device program
# ---------------------------------------------------------------------------

def build_program(b_shard: int = B_SHARD, with_b2: bool = False) -> bass.Bass:
    assert b_shard % P == 0
    n_tiles = b_shard // P

    nc = bacc.Bacc("TRN2", target_bir_lowering=False, debug=False,
                   num_devices=N_CORES)
    x_d = nc.dram_tensor("x", [b_shard, D], F32, kind="ExternalInput")
    weff_d = nc.dram_tensor("weff", [P, 128], F32, kind="ExternalInput")
    hflat_d = nc.dram_tensor("hflat", [256, 128], F32, kind="ExternalInput")
    w2t_d = nc.dram_tensor("w2t", [P, D], F32, kind="ExternalInput")
    b1_d = nc.dram_tensor("b1c", [P, 1], F32, kind="ExternalInput")
    ident_d = nc.dram_tensor("ident", [P, P], F32, kind="ExternalInput")
    if with_b2:
        b2_d = nc.dram_tensor("b2r", [1, D], F32, kind="ExternalInput")
    out_d = nc.dram_tensor("out", [b_shard, D], F32, kind="ExternalOutput")

    x_ap = x_d.ap()
    out_ap = out_d.ap()
    AX = mybir.AxisListType.X
    mult = mybir.AluOpType.mult
    add = mybir.AluOpType.add
    AF = mybir.ActivationFunctionType

    with tile.TileContext(nc) as tc, ExitStack() as ctx:
        cpool = ctx.enter_context(tc.tile_pool(name="consts", bufs=1))
        weff_sb = cpool.tile([P, 128], F32)
        nc.sync.dma_start(weff_sb[:], weff_d.ap())
        hfa_sb = cpool.tile([P, 128], F32)
        nc.sync.dma_start(hfa_sb[:], hflat_d.ap()[0:128, :])
        hfb_sb = cpool.tile([P, 128], F32)
        nc.sync.dma_start(hfb_sb[:], hflat_d.ap()[128:256, :])
        w2t_sb = cpool.tile([P, D], F32)
        nc.sync.dma_start(w2t_sb[:], w2t_d.ap())
        b1_sb = cpool.tile([P, 1], F32)
        nc.sync.dma_start(b1_sb[:], b1_d.ap())
        id_sb = cpool.tile([P, P], F32)
        nc.sync.dma_start(id_sb[:], ident_d.ap())
        if with_b2:
            b2_sb = cpool.tile([1, D], F32)
            nc.sync.dma_start(b2_sb[:], b2_d.ap())
            ones_sb = cpool.tile([1, P], F32)
            nc.gpsimd.memset(ones_sb[:], 1.0)
        halfpi_sb = cpool.tile([P, 1], F32)
        nc.gpsimd.memset(halfpi_sb[:], HALF_PI)
        zero_sb = cpool.tile([P, 1], F32)
        nc.gpsimd.memset(zero_sb[:], 0.0)

        xpool = ctx.enter_context(tc.tile_pool(name="xin", bufs=3))
        opool = ctx.enter_context(tc.tile_pool(name="osb", bufs=3))
        wpool = ctx.enter_context(tc.tile_pool(name="work", bufs=2))
        pps = ctx.enter_context(tc.tile_pool(name="ps_small", bufs=2, space="PSUM"))
        ppo = ctx.enter_context(tc.tile_pool(name="ps_out", bufs=2, space="PSUM"))

        for t in range(n_tiles):
            rows = slice(t * P, (t + 1) * P)
            xt = xpool.tile([P, D], F32)
            nc.sync.dma_start(xt[:], x_ap[rows, :])

            # ---- features F [128, 32] ----
            F = wpool.tile([P, 32], F32)
            nc.gpsimd.memset(F[:, 27:28], 1.0)
            nc.gpsimd.memset(F[:, 28:32], 0.0)
            xa = xt[:]
            x3 = xa.rearrange("p (c n) -> p c n", c=3)
            x4 = xa.rearrange("p (c h w) -> p c h w", c=3, h=32)
            nc.vector.reduce_sum(F[:, 0:3], x3, axis=AX)                    # totals
            nc.vector.reduce_sum(F[:, 3:6], x3[:, :, 0:32], axis=AX)        # row 0
            nc.vector.reduce_sum(F[:, 6:9], x3[:, :, 992:1024], axis=AX)    # row 31
            nc.vector.reduce_sum(F[:, 9:12], x4[:, :, :, 0], axis=AX)       # col 0
            nc.vector.reduce_sum(F[:, 12:15], x4[:, :, :, 31], axis=AX)     # col 31
            nc.vector.tensor_copy(
                F[:, 15:27].rearrange("p (i r c) -> p i r c", i=3, r=2),
                x4[:, :, 0:32:31, 0:32:31],
            )

            # ---- pooled [128, 4] via fused multiply-reduce ----
            pooled = wpool.tile([P, 4], F32)
            scr = wpool.tile([P, 32], F32)
            for o in range(4):
                nc.vector.scalar_tensor_tensor(
                    out=scr[:], in0=F[:], scalar=1.0,
                    in1=weff_sb[:, o * 32:(o + 1) * 32],
                    op0=mult, op1=mult,
                    accum_out=pooled[:, o:o + 1],
                )

            # ---- cos/sin of pooled/2 ----
            cs = wpool.tile([P, 8], F32)
            nc.scalar.activation(cs[:, 0:4], pooled[:], AF.Sin,
                                 bias=halfpi_sb[:, 0:1], scale=0.5)  # cos
            nc.scalar.activation(cs[:, 4:8], pooled[:], AF.Sin,
                                 bias=zero_sb[:, 0:1], scale=0.5)    # sin

            # ---- product state s16 and outer products P2 ----
            t2 = wpool.tile([P, 4], F32)
            nc.vector.tensor_mul(
                t2[:].rearrange("p (a b) -> p a b", a=2),
                cs[:, 0:8:4].unsqueeze(-1).broadcast_to((P, 2, 2)),
                cs[:, 1:8:4].unsqueeze(1).broadcast_to((P, 2, 2)))
            t4 = wpool.tile([P, 8], F32)
            nc.vector.tensor_mul(
                t4[:].rearrange("p (a b) -> p a b", a=4),
                t2[:].unsqueeze(-1).broadcast_to((P, 4, 2)),
                cs[:, 2:8:4].unsqueeze(1).broadcast_to((P, 4, 2)))
            s16 = wpool.tile([P, DIM], F32)
            nc.vector.tensor_mul(
                s16[:].rearrange("p (a b) -> p a b", a=8),
                t4[:].unsqueeze(-1).broadcast_to((P, 8, 2)),
                cs[:, 3:8:4].unsqueeze(1).broadcast_to((P, 8, 2)))
            P2 = wpool.tile([P, 256], F32)
            nc.vector.tensor_mul(
                P2[:].rearrange("p (k l) -> p k l", k=DIM),
                s16[:].unsqueeze(-1).broadcast_to((P, DIM, DIM)),
                s16[:].unsqueeze(1).broadcast_to((P, DIM, DIM)))

            # ---- transpose P2 -> [256 kk', 128 b] and hpre = H @ P2T ----
            p2t = pps.tile([P, 256], F32)
            nc.tensor.transpose(p2t[:, 0:128], P2[:, 0:128], id_sb[:])
            nc.tensor.transpose(p2t[:, 128:256], P2[:, 128:256], id_sb[:])
            p2sb = wpool.tile([P, 256], F32)
            nc.scalar.copy(p2sb[:, 0:128], p2t[:, 0:128])
            nc.scalar.copy(p2sb[:, 128:256], p2t[:, 128:256])

            hpre = pps.tile([P, P], F32)
            nc.tensor.matmul(hpre[:], hfa_sb[:], p2sb[:, 0:128],
                             start=True, stop=False)
            nc.tensor.matmul(hpre[:], hfb_sb[:], p2sb[:, 128:256],
                             start=False, stop=True)

            hT = wpool.tile([P, P], F32)
            nc.scalar.activation(hT[:], hpre[:], AF.Relu, bias=b1_sb[:, 0:1],
                                 scale=1.0)

            # ---- out tile = relu(h)^T-matmul against w2^T (+ b2) ----
            osb = opool.tile([P, D], F32)
            for c in range(3):
                ops = ppo.tile([P, 1024], F32)
                for half in range(2):
                    col0 = c * 1024 + half * 512
                    psl = slice(half * 512, half * 512 + 512)
                    if with_b2:
                        nc.tensor.matmul(ops[:, psl], hT[:],
                                         w2t_sb[:, col0:col0 + 512],
                                         start=True, stop=False)
                        nc.tensor.matmul(ops[:, psl], ones_sb[:],
                                         b2_sb[:, col0:col0 + 512],
                                         start=False, stop=True)
                    else:
                        nc.tensor.matmul(ops[:, psl], hT[:],
                                         w2t_sb[:, col0:col0 + 512],
                                         start=True, stop=True)
                nc.scalar.copy(osb[:, c * 1024:(c + 1) * 1024], ops[:])
            nc.sync.dma_start(out_ap[rows, :], osb[:])

    nc.compile()
    return nc


# ---------------------------------------------------------------------------
# Host entry point
# ---------------------------------------------------------------------------

def _host_consts(conv_w, conv_b, q_weights, w1, b1, w2, b2):
    weff = _feature_weights(np.asarray(conv_w), np.asarray(conv_b))  # [32, 4]
    weff_rep = np.ascontiguousarray(
        np.tile(weff.T.reshape(1, 128), (P, 1))).astype(np.float32)  # [128,128]
    H = _quadratic_forms(np.asarray(q_weights), np.asarray(w1))      # [128,16,16]
    hflat = np.ascontiguousarray(
        H.transpose(1, 2, 0).reshape(256, 128)).astype(np.float32)
    w2t = np.ascontiguousarray(np.asarray(w2).T).astype(np.float32)  # [128,3072]
    b1c = np.ascontiguousarray(np.asarray(b1).reshape(P, 1)).astype(np.float32)
    ident = np.eye(P, dtype=np.float32)
    consts = {"weff": weff_rep, "hflat": hflat, "w2t": w2t, "b1c": b1c,
              "ident": ident}
    with_b2 = bool(np.any(np.asarray(b2)))
    if with_b2:
        consts["b2r"] = np.ascontiguousarray(
            np.asarray(b2).reshape(1, D)).astype(np.float32)
    return consts, with_b2


_PROGRAM_CACHE: dict = {}


def _get_program(b_shard: int, with_b2: bool) -> bass.Bass:
    key = (b_shard, with_b2)
    if key not in _PROGRAM_CACHE:
        _PROGRAM_CACHE[key] = build_program(b_shard, with_b2)
    return _PROGRAM_CACHE[key]


def run(x, conv_w, conv_b, q_weights, w1, b1, w2, b2, trace=False, **kw):
    x = np.ascontiguousarray(np.asarray(x), dtype=np.float32)
    B = x.shape[0]
    assert B % N_CORES == 0
    b_shard = B // N_CORES
    consts, with_b2 = _host_consts(conv_w, conv_b, q_weights, w1, b1, w2, b2)
    nc = _get_program(b_shard, with_b2)
    shards = x.reshape(N_CORES, b_shard, D)
    in_maps = [{"x": np.ascontiguousarray(shards[i]), **consts}
               for i in range(N_CORES)]
    res = run_bass_kernel_spmd(nc, in_maps, list(range(N_CORES)),
                               trace=trace, **kw)
    out = np.concatenate([res.results[i]["out"] for i in range(N_CORES)], axis=0)
    return out.reshape(B, 3, 32, 32).astype(np.float32), res


def kernel(x, conv_w, conv_b, q_weights, w1, b1, w2, b2):
    out, _ = run(x, conv_w, conv_b, q_weights, w1, b1, w2, b2)
    return out


# revision 10
# speedup vs baseline: 29.7934x; 29.7934x over previous
"""Trainium2 Bass kernel for nn_DenoiseQNN (conv -> global avgpool -> 4-qubit
quantum circuit -> MLP decoder), data-parallel over 8 NeuronCores.

Math folding (validated against the jax reference on host):
  * conv(3->4, 3x3, SAME) followed by global mean is LINEAR in x, and depends
    on x only through 27 per-sample scalars: per input channel the total sum,
    the 4 border row/col sums, and the 4 corner pixels (inclusion-exclusion
    over the 9 kernel taps). pooled = F @ Weff (+ conv_b via a const-1
    feature).
  * the quantum state after the per-sample RY embedding layer is the real
    product state s_ry[j] = prod_w (cos(p_w/2) if bit_w(j)==0 else sin(p_w/2)).
    The remaining RX layers + CNOT rings use only the shared q_weights, so
    they form a fixed complex 16x16 matrix M. <Z_w> = s^T G_w s with
    G_w = Re(M)^T diag(Z_w) Re(M) + Im(M)^T diag(Z_w) Im(M), and the first MLP
    layer folds in: hpre_m = s^T H_m s, H_m = sum_w w1[m,w] G_w.
  * out = relu(hpre + b1) @ w2.T + b2 -> [B, 3072].

Device pipeline per 128-sample tile (batch on partitions):
  DMA x tile [128, 3072] -> DVE reductions build F [128, 32] -> 4 fused
  multiply-reduce ops give pooled [128, 4] -> ScalarE Sin activations give
  cos/sin -> DVE broadcast-multiplies build s16 and the outer products
  P2 [128, 256] -> TensorE transposes P2 -> two matmuls against H (lhsT)
  give hpre [128m, 128b] -> ScalarE relu+bias -> TensorE [128,128]x[128,3072]
  matmul against w2^T -> ScalarE PSUM->SBUF copy -> DMA out.

Traffic is 2 x 201 MB (read x, write out) over 8 cores; everything else is
tiny, so the kernel is HBM-bound as intended.
"""

import math
from contextlib import ExitStack

import numpy as np

import concourse.bass as bass
import concourse.mybir as mybir
import concourse.tile as tile
from concourse import bacc
from concourse.bass_utils import run_bass_kernel_spmd

N_CORES = 8
B_FULL = 16384
B_SHARD = B_FULL // N_CORES  # 2048
P = 128
D = 3072  # 3*32*32
N_QUBITS = 4
DIM = 16
F32 = mybir.dt.float32
HALF_PI = math.pi / 2.0


# ---------------------------------------------------------------------------
# Host-side parameter folding
# ---------------------------------------------------------------------------

def _feature_weights(conv_w: np.ndarray, conv_b: np.ndarray) -> np.ndarray:
    """Weff [32, 4]: pooled = F @ Weff with the device feature layout
    F = [S(3), R0(3), R31(3), C0(3), C31(3), corners(3x2x2), 1, pad(4)]."""
    W = np.zeros((32, N_QUBITS), np.float64)
    cw = conv_w.astype(np.float64)
    for o in range(N_QUBITS):
        for i in range(3):
            for dh in range(3):
                for dw in range(3):
                    c = cw[o, i, dh, dw]
                    W[0 + i, o] += c                     # total sum
                    if dh == 2:
                        W[3 + i, o] -= c                 # row 0 excluded
                    if dh == 0:
                        W[6 + i, o] -= c                 # row 31 excluded
                    if dw == 2:
                        W[9 + i, o] -= c                 # col 0 excluded
                    if dw == 0:
                        W[12 + i, o] -= c                # col 31 excluded
                    # corners (i, r, c): r,c in {0,31}
                    if (dh, dw) == (2, 2):
                        W[15 + i * 4 + 0, o] += c        # x[0,0]
                    if (dh, dw) == (2, 0):
                        W[15 + i * 4 + 1, o] += c        # x[0,31]
                    if (dh, dw) == (0, 2):
                        W[15 + i * 4 + 2, o] += c        # x[31,0]
                    if (dh, dw) == (0, 0):
                        W[15 + i * 4 + 3, o] += c        # x[31,31]
    W /= 1024.0
    W[27, :] = conv_b.astype(np.float64)
    return W.astype(np.float32)


def _quantum_fixed_matrix(q_weights: np.ndarray) -> np.ndarray:
    """M [16,16] complex: the fixed post-RY linear map (RX layers + CNOT rings)."""
    M = np.eye(DIM, dtype=np.complex128)

    def apply_1q(Mat, U, wire):
        T = Mat.reshape(2**wire, 2, 2 ** (N_QUBITS - 1 - wire), DIM)
        T = np.einsum("ij,ajcb->aicb", U, T)
        return T.reshape(DIM, DIM)

    idx = np.arange(DIM)
    perms = []
    for c in range(N_QUBITS):
        t = (c + 1) % N_QUBITS
        mc = 1 << (N_QUBITS - 1 - c)
        mt = 1 << (N_QUBITS - 1 - t)
        perms.append(np.where(idx & mc, idx ^ mt, idx))

    for layer in range(q_weights.shape[0]):
        for w in range(N_QUBITS):
            th = float(q_weights[layer, w]) * 0.5
            cc = np.cos(th)
            ss = -1j * np.sin(th)
            M = apply_1q(M, np.array([[cc, ss], [ss, cc]]), w)
        for w in range(N_QUBITS):
            M = M[perms[w], :]
    return M


def _quadratic_forms(q_weights: np.ndarray, w1: np.ndarray) -> np.ndarray:
    """H [128, 16, 16]: hpre[m] = s^T H_m s."""
    M = _quantum_fixed_matrix(q_weights)
    bits = (np.arange(DIM)[None, :] >> (N_QUBITS - 1 - np.arange(N_QUBITS)[:, None])) & 1
    Z = 1.0 - 2.0 * bits
    Re, Im = M.real, M.imag
    G = np.einsum("wj,jk,jl->wkl", Z, Re, Re) + np.einsum("wj,jk,jl->wkl", Z, Im, Im)
    return np.einsum("mw,wkl->mkl", w1.astype(np.float64), G).astype(np.float32)


# ---------------------------------------------------------------------------
# Device program
# ---------------------------------------------------------------------------

def build_program(b_shard: int = B_SHARD, with_b2: bool = False,
                  repeats: int = 1) -> bass.Bass:
    """repeats>1 re-runs the whole tile loop (same I/O) — used only for
    slope-based timing on hardware; output is identical."""
    assert b_shard % P == 0
    n_tiles = b_shard // P

    nc = bacc.Bacc("TRN2", target_bir_lowering=False, debug=False,
                   num_devices=N_CORES)
    x_d = nc.dram_tensor("x", [b_shard, D], F32, kind="ExternalInput")
    weff_d = nc.dram_tensor("weff", [P, 128], F32, kind="ExternalInput")
    hflat_d = nc.dram_tensor("hflat", [256, 128], F32, kind="ExternalInput")
    w2t_d = nc.dram_tensor("w2t", [P, D], F32, kind="ExternalInput")
    b1_d = nc.dram_tensor("b1c", [P, 1], F32, kind="ExternalInput")
    ident_d = nc.dram_tensor("ident", [P, P], F32, kind="ExternalInput")
    if with_b2:
        b2_d = nc.dram_tensor("b2r", [1, D], F32, kind="ExternalInput")
    out_d = nc.dram_tensor("out", [b_shard, D], F32, kind="ExternalOutput")

    x_ap = x_d.ap()
    out_ap = out_d.ap()
    AX = mybir.AxisListType.X
    mult = mybir.AluOpType.mult
    add = mybir.AluOpType.add
    AF = mybir.ActivationFunctionType

    with tile.TileContext(nc) as tc, ExitStack() as ctx:
        cpool = ctx.enter_context(tc.tile_pool(name="consts", bufs=1))
        weff_sb = cpool.tile([P, 128], F32)
        nc.sync.dma_start(weff_sb[:], weff_d.ap())
        hfa_sb = cpool.tile([P, 128], F32)
        nc.sync.dma_start(hfa_sb[:], hflat_d.ap()[0:128, :])
        hfb_sb = cpool.tile([P, 128], F32)
        nc.sync.dma_start(hfb_sb[:], hflat_d.ap()[128:256, :])
        w2t_sb = cpool.tile([P, D], F32)
        nc.sync.dma_start(w2t_sb[:], w2t_d.ap())
        b1_sb = cpool.tile([P, 1], F32)
        nc.sync.dma_start(b1_sb[:], b1_d.ap())
        id_sb = cpool.tile([P, P], F32)
        nc.sync.dma_start(id_sb[:], ident_d.ap())
        if with_b2:
            b2_sb = cpool.tile([1, D], F32)
            nc.sync.dma_start(b2_sb[:], b2_d.ap())
            ones_sb = cpool.tile([1, P], F32)
            nc.gpsimd.memset(ones_sb[:], 1.0)
        halfpi_sb = cpool.tile([P, 1], F32)
        nc.gpsimd.memset(halfpi_sb[:], HALF_PI)
        zero_sb = cpool.tile([P, 1], F32)
        nc.gpsimd.memset(zero_sb[:], 0.0)

        xpool = ctx.enter_context(tc.tile_pool(name="xin", bufs=3))
        opool = ctx.enter_context(tc.tile_pool(name="osb", bufs=3))
        wpool = ctx.enter_context(tc.tile_pool(name="work", bufs=2))
        pps = ctx.enter_context(tc.tile_pool(name="ps_small", bufs=2, space="PSUM"))
        ppo = ctx.enter_context(tc.tile_pool(name="ps_out", bufs=2, space="PSUM"))

        for t in range(n_tiles * repeats):
            t = t % n_tiles
            rows = slice(t * P, (t + 1) * P)
            xt = xpool.tile([P, D], F32)
            nc.sync.dma_start(xt[:], x_ap[rows, :])

            # ---- features F [128, 32] ----
            F = wpool.tile([P, 32], F32)
            nc.gpsimd.memset(F[:, 27:28], 1.0)
            nc.gpsimd.memset(F[:, 28:32], 0.0)
            xa = xt[:]
            x3 = xa.rearrange("p (c n) -> p c n", c=3)
            x4 = xa.rearrange("p (c h w) -> p c h w", c=3, h=32)
            nc.vector.reduce_sum(F[:, 0:3], x3, axis=AX)                    # totals
            nc.vector.reduce_sum(F[:, 3:6], x3[:, :, 0:32], axis=AX)        # row 0
            nc.vector.reduce_sum(F[:, 6:9], x3[:, :, 992:1024], axis=AX)    # row 31
            nc.vector.reduce_sum(F[:, 9:12], x4[:, :, :, 0], axis=AX)       # col 0
            nc.vector.reduce_sum(F[:, 12:15], x4[:, :, :, 31], axis=AX)     # col 31
            nc.vector.tensor_copy(
                F[:, 15:27].rearrange("p (i r c) -> p i r c", i=3, r=2),
                x4[:, :, 0:32:31, 0:32:31],
            )

            # ---- pooled [128, 4] via fused multiply-reduce ----
            pooled = wpool.tile([P, 4], F32)
            scr = wpool.tile([P, 32], F32)
            for o in range(4):
                nc.vector.scalar_tensor_tensor(
                    out=scr[:], in0=F[:], scalar=1.0,
                    in1=weff_sb[:, o * 32:(o + 1) * 32],
                    op0=mult, op1=mult,
                    accum_out=pooled[:, o:o + 1],
                )

            # ---- cos/sin of pooled/2 ----
            cs = wpool.tile([P, 8], F32)
            nc.scalar.activation(cs[:, 0:4], pooled[:], AF.Sin,
                                 bias=halfpi_sb[:, 0:1], scale=0.5)  # cos
            nc.scalar.activation(cs[:, 4:8], pooled[:], AF.Sin,
                                 bias=zero_sb[:, 0:1], scale=0.5)    # sin

            # ---- product state s16 and outer products P2 ----
            t2 = wpool.tile([P, 4], F32)
            nc.vector.tensor_mul(
                t2[:].rearrange("p (a b) -> p a b", a=2),
                cs[:, 0:8:4].unsqueeze(-1).broadcast_to((P, 2, 2)),
                cs[:, 1:8:4].unsqueeze(1).broadcast_to((P, 2, 2)))
            t4 = wpool.tile([P, 8], F32)
            nc.vector.tensor_mul(
                t4[:].rearrange("p (a b) -> p a b", a=4),
                t2[:].unsqueeze(-1).broadcast_to((P, 4, 2)),
                cs[:, 2:8:4].unsqueeze(1).broadcast_to((P, 4, 2)))
            s16 = wpool.tile([P, DIM], F32)
            nc.vector.tensor_mul(
                s16[:].rearrange("p (a b) -> p a b", a=8),
                t4[:].unsqueeze(-1).broadcast_to((P, 8, 2)),
                cs[:, 3:8:4].unsqueeze(1).broadcast_to((P, 8, 2)))
            P2 = wpool.tile([P, 256], F32)
            nc.vector.tensor_mul(
                P2[:].rearrange("p (k l) -> p k l", k=DIM),
                s16[:].unsqueeze(-1).broadcast_to((P, DIM, DIM)),
                s16[:].unsqueeze(1).broadcast_to((P, DIM, DIM)))

            # ---- transpose P2 -> [256 kk', 128 b] and hpre = H @ P2T ----
            p2t = pps.tile([P, 256], F32)
            nc.tensor.transpose(p2t[:, 0:128], P2[:, 0:128], id_sb[:])
            nc.tensor.transpose(p2t[:, 128:256], P2[:, 128:256], id_sb[:])
            p2sb = wpool.tile([P, 256], F32)
            nc.scalar.copy(p2sb[:, 0:128], p2t[:, 0:128])
            nc.scalar.copy(p2sb[:, 128:256], p2t[:, 128:256])

            hpre = pps.tile([P, P], F32)
            nc.tensor.matmul(hpre[:], hfa_sb[:], p2sb[:, 0:128],
                             start=True, stop=False)
            nc.tensor.matmul(hpre[:], hfb_sb[:], p2sb[:, 128:256],
                             start=False, stop=True)

            hT = wpool.tile([P, P], F32)
            nc.scalar.activation(hT[:], hpre[:], AF.Relu, bias=b1_sb[:, 0:1],
                                 scale=1.0)

            # ---- out tile = relu(h)^T-matmul against w2^T (+ b2) ----
            osb = opool.tile([P, D], F32)
            for c in range(3):
                ops = ppo.tile([P, 1024], F32)
                for half in range(2):
                    col0 = c * 1024 + half * 512
                    psl = slice(half * 512, half * 512 + 512)
                    if with_b2:
                        nc.tensor.matmul(ops[:, psl], hT[:],
                                         w2t_sb[:, col0:col0 + 512],
                                         start=True, stop=False)
                        nc.tensor.matmul(ops[:, psl], ones_sb[:],
                                         b2_sb[:, col0:col0 + 512],
                                         start=False, stop=True)
                    else:
                        nc.tensor.matmul(ops[:, psl], hT[:],
                                         w2t_sb[:, col0:col0 + 512],
                                         start=True, stop=True)
                nc.scalar.copy(osb[:, c * 1024:(c + 1) * 1024], ops[:])
            nc.sync.dma_start(out_ap[rows, :], osb[:])

    nc.compile()
    return nc


# ---------------------------------------------------------------------------
# Host entry point
# ---------------------------------------------------------------------------

def _host_consts(conv_w, conv_b, q_weights, w1, b1, w2, b2):
    weff = _feature_weights(np.asarray(conv_w), np.asarray(conv_b))  # [32, 4]
    weff_rep = np.ascontiguousarray(
        np.tile(weff.T.reshape(1, 128), (P, 1))).astype(np.float32)  # [128,128]
    H = _quadratic_forms(np.asarray(q_weights), np.asarray(w1))      # [128,16,16]
    hflat = np.ascontiguousarray(
        H.transpose(1, 2, 0).reshape(256, 128)).astype(np.float32)
    w2t = np.ascontiguousarray(np.asarray(w2).T).astype(np.float32)  # [128,3072]
    b1c = np.ascontiguousarray(np.asarray(b1).reshape(P, 1)).astype(np.float32)
    ident = np.eye(P, dtype=np.float32)
    consts = {"weff": weff_rep, "hflat": hflat, "w2t": w2t, "b1c": b1c,
              "ident": ident}
    with_b2 = bool(np.any(np.asarray(b2)))
    if with_b2:
        consts["b2r"] = np.ascontiguousarray(
            np.asarray(b2).reshape(1, D)).astype(np.float32)
    return consts, with_b2


_PROGRAM_CACHE: dict = {}


def _get_program(b_shard: int, with_b2: bool, repeats: int = 1) -> bass.Bass:
    key = (b_shard, with_b2, repeats)
    if key not in _PROGRAM_CACHE:
        _PROGRAM_CACHE[key] = build_program(b_shard, with_b2, repeats)
    return _PROGRAM_CACHE[key]


def run(x, conv_w, conv_b, q_weights, w1, b1, w2, b2, trace=False, **kw):
    x = np.ascontiguousarray(np.asarray(x), dtype=np.float32)
    B = x.shape[0]
    assert B % N_CORES == 0
    b_shard = B // N_CORES
    consts, with_b2 = _host_consts(conv_w, conv_b, q_weights, w1, b1, w2, b2)
    nc = _get_program(b_shard, with_b2)
    shards = x.reshape(N_CORES, b_shard, D)
    in_maps = [{"x": np.ascontiguousarray(shards[i]), **consts}
               for i in range(N_CORES)]
    res = run_bass_kernel_spmd(nc, in_maps, list(range(N_CORES)),
                               trace=trace, **kw)
    out = np.concatenate([res.results[i]["out"] for i in range(N_CORES)], axis=0)
    return out.reshape(B, 3, 32, 32).astype(np.float32), res


def kernel(x, conv_w, conv_b, q_weights, w1, b1, w2, b2):
    out, _ = run(x, conv_w, conv_b, q_weights, w1, b1, w2, b2)
    return out
